# revision 1
# baseline (speedup 1.0000x reference)
"""Causal single-head attention (B=16, T=2048, C=1024, H=64) on 8 TRN2 NeuronCores.

Strategy:
- Data-parallel over batch: 2 batches per core, weights replicated.
- Host passes x pre-transposed per batch (xT: [C, T]) so projections can
  contract over C on the PE partition dim with full-rate fp32r matmuls.
- Projections: packed [Wq.T | Wk.T] stationary -> QKT [128, T] (rows 0:64 = Q^T,
  64:128 = K^T); Wv.T -> VT [64, T]; V^T transposed to V natural via PE transpose.
- Attention computed transposed: S^T[k,q] = KT_blk.T @ QT (N=512 full rate),
  P' = exp(0.125*S^T) on ACT (no max subtraction needed: scores are O(1)),
  causal mask via precomputed 0/1 mask multiply on diagonal chunks,
  O'^T[65,q] = [V|1].T @ P' accumulated over k-chunks; row 64 = softmax denom.
- Final PE transpose back to natural layout, reciprocal + scale, DMA out.
"""
import os
import sys

for _p in ("/opt/trn_rl_repo", "/root/.axon_site/_ro/trn_rl_repo"):
    if os.path.isdir(_p) and _p not in sys.path:
        sys.path.insert(0, _p)

import numpy as np
import ml_dtypes
import concourse.bacc as bacc
import concourse.mybir as mybir
from concourse.tile import TileContext
from concourse import bass_utils

F32 = mybir.dt.float32
F32R = mybir.dt.float32r
BF16 = mybir.dt.bfloat16
EXP = mybir.ActivationFunctionType.Exp

B, T, C, H = 16, 2048, 1024, 64
NCORES = 8
BPC = B // NCORES          # batches per core
NTS = T // 512             # 4 t/q slices of 512
NCH = C // 128             # 8 contraction chunks
NKC = T // 128             # 16 k chunks

LAST_EXEC_TIME_NS = None
LAST_RESULTS = None


def build():
    nc = bacc.Bacc(trn_type="TRN2")
    xt = nc.dram_tensor("xt", [BPC, C, T], BF16, kind="ExternalInput")
    wqk = nc.dram_tensor("wqk", [C, 128], BF16, kind="ExternalInput")
    wv = nc.dram_tensor("wv", [C, H], BF16, kind="ExternalInput")
    mask = nc.dram_tensor("mask", [128, 896], BF16, kind="ExternalInput")
    ident = nc.dram_tensor("ident", [128, 128], F32R, kind="ExternalInput")
    ident_bf = nc.dram_tensor("ident_bf", [64, 64], BF16, kind="ExternalInput")
    ones = nc.dram_tensor("ones", [128, NKC * 32], BF16, kind="ExternalInput")
    y = nc.dram_tensor("y", [BPC, T, H], F32, kind="ExternalOutput")

    with TileContext(nc) as tc:
        with tc.tile_pool(name="const", bufs=1) as const, \
             tc.tile_pool(name="xpool", bufs=3) as xpool, \
             tc.tile_pool(name="qktp", bufs=2) as qktp, \
             tc.tile_pool(name="vtp", bufs=2) as vtp, \
             tc.tile_pool(name="ktp", bufs=2) as ktp, \
             tc.tile_pool(name="vbigp", bufs=2) as vbigp, \
             tc.tile_pool(name="ptp", bufs=6) as ptp, \
             tc.tile_pool(name="osbp", bufs=3) as osbp, \
             tc.tile_pool(name="yp", bufs=8) as yp, \
             tc.tile_pool(name="ps512", bufs=4, space="PSUM") as ps512, \
             tc.tile_pool(name="pssm", bufs=4, space="PSUM") as pssm:

            wqk_sb = []
            wv_sb = []
            for c in range(NCH):
                wq_t = const.tile([128, 128], BF16, name=f"wqk{c}")
                nc.scalar.dma_start(wq_t[:], wqk[128 * c:128 * (c + 1), :])
                wqk_sb.append(wq_t)
                wv_t = const.tile([128, H], BF16, name=f"wv{c}")
                nc.scalar.dma_start(wv_t[:], wv[128 * c:128 * (c + 1), :])
                wv_sb.append(wv_t)
            mask_sb = const.tile([128, 896], BF16, name="mask_sb")
            nc.scalar.dma_start(mask_sb[:], mask[:])
            id_sb = const.tile([128, 128], F32R, name="id_sb")
            nc.scalar.dma_start(id_sb[:], ident[:])
            id_bf = const.tile([64, 64], BF16, name="id_bf")
            nc.scalar.dma_start(id_bf[:], ident_bf[:])

            for b in range(BPC):
                qkt = qktp.tile([128, T], BF16, name="qkt", tag="qkt")
                vt = vtp.tile([64, T], BF16, name="vt", tag="vt")
                kt = ktp.tile([64, T], BF16, name="kt", tag="kt")
                vbig = vbigp.tile([128, NKC * 96], BF16, name="vbig", tag="vbig")
                vcols = vbig[:].rearrange("p (i c) -> p i c", c=96)[:, :, H:96]
                nc.gpsimd.dma_start(vcols, ones[:].rearrange("p (i c) -> p i c", c=32))

                # ---- fused pipeline: proj(ts) -> V-transpose(ts) -> attn(j=ts) ----
                # causality: attention slice j only reads k-chunks i <= 4j+3,
                # i.e. data from t-slices <= ts, so each slice's attention can
                # run as soon as its own projections land.
                for ts in range(NTS):
                    if True:
                        xgs = []
                        for g in range(2):
                            xg = xpool.tile([128, 4 * 512], BF16, name=f"xg{g}",
                                            tag=f"xg{g}")
                            src = xt[b, 512 * g:512 * (g + 1),
                                     512 * ts:512 * (ts + 1)].rearrange(
                                         "(a p) t -> p a t", p=128)
                            dst = xg[:].rearrange("p (a t) -> p a t", t=512)
                            eng = nc.sync if g == 0 else nc.scalar
                            eng.dma_start(dst, src)
                            xgs.append(xg)
                        xts = [xgs[c // 4][:, 512 * (c % 4):512 * (c % 4 + 1)]
                               for c in range(NCH)]
                    qk_ps = ps512.tile([128, 512], F32, name="qk_ps", tag="ps512")
                    for c in range(NCH):
                        nc.tensor.matmul(qk_ps[:], wqk_sb[c][:], xts[c],
                                         start=(c == 0), stop=(c == NCH - 1))
                    nc.vector.tensor_copy(qkt[:, 512 * ts:512 * (ts + 1)], qk_ps[:])
                    nc.sync.dma_start(kt[:, 512 * ts:512 * (ts + 1)],
                                      qkt[64:128, 512 * ts:512 * (ts + 1)])
                    v_ps = pssm.tile([64, 512], F32, name="v_ps", tag="pssm")
                    for c in range(NCH):
                        nc.tensor.matmul(v_ps[:], wv_sb[c][:], xts[c],
                                         start=(c == 0), stop=(c == NCH - 1))
                    nc.vector.tensor_copy(vt[:, 512 * ts:512 * (ts + 1)], v_ps[:])

                    for i in range(4 * ts, 4 * ts + 4):
                        vtr_ps = pssm.tile([128, H], BF16, name="vtr_ps", tag="pssm")
                        nc.tensor.transpose(vtr_ps[:], vt[:, 128 * i:128 * (i + 1)],
                                            id_bf[:])
                        nc.vector.tensor_copy(vbig[:, 96 * i:96 * i + H], vtr_ps[:])

                    # attention for q-slice j == ts, PV pipelined 1 chunk behind S
                    j = ts
                    nck = 4 * j + 4
                    o_ps = pssm.tile([96, 512], F32, name="o_ps", tag="pssm")
                    pts = []
                    for i in range(nck):
                        d = i - 4 * j
                        o = 128 * d if d > 0 else 0   # causal col offset in slice
                        w = 512 - o
                        s_ps = ps512.tile([128, 512], F32, name="s_ps", tag="ps512")
                        nc.tensor.matmul(s_ps[:, o:512], kt[:, 128 * i:128 * (i + 1)],
                                         qkt[0:64, 512 * j + o:512 * (j + 1)],
                                         start=True, stop=True)
                        p_t = ptp.tile([128, 512], BF16, name="p_t", tag="pt")
                        nc.scalar.activation(p_t[:, o:512], s_ps[:, o:512], EXP,
                                             scale=0.125)
                        if d >= 0:
                            nc.vector.tensor_mul(
                                p_t[:, o:512], p_t[:, o:512],
                                mask_sb[:, 384:384 + w])
                        pts.append((p_t, o))
                        if i > 0:
                            pp, po = pts[i - 1]
                            nc.tensor.matmul(o_ps[:, po:512],
                                             vbig[:, 96 * (i - 1):96 * i],
                                             pp[:, po:512], start=(i - 1 == 0),
                                             stop=False)
                    pp, po = pts[nck - 1]
                    nc.tensor.matmul(o_ps[:, po:512],
                                     vbig[:, 96 * (nck - 1):96 * nck],
                                     pp[:, po:512], start=(nck == 1),
                                     stop=True)
                    o_sb = osbp.tile([96, 512], F32R, name="o_sb", tag="osb")
                    nc.vector.tensor_copy(o_sb[:], o_ps[:])
                    for s in range(4):
                        f_ps = pssm.tile([128, 96], F32R, name="f_ps", tag="pssm")
                        nc.tensor.transpose(f_ps[:], o_sb[:, 128 * s:128 * (s + 1)],
                                            id_sb[0:96, 0:96])
                        rec = yp.tile([128, 1], F32, name="rec", tag="rec")
                        nc.vector.reciprocal(rec[:], f_ps[:, H:H + 1])
                        y_t = yp.tile([128, H], F32, name="y_t", tag="yt")
                        nc.vector.tensor_scalar_mul(y_t[:], f_ps[:, 0:H], rec[:])
                        q0 = 512 * j + 128 * s
                        nc.gpsimd.dma_start(y[b, q0:q0 + 128, :], y_t[:])

    nc.finalize()
    return nc


_NC_CACHE = None


def _get_nc():
    global _NC_CACHE
    if _NC_CACHE is None:
        _NC_CACHE = build()
    return _NC_CACHE


def _make_mask():
    # mask[p, m] = 1.0 iff (m - 384) >= p ; diagonal chunk d uses cols
    # [384-128d : 896-128d) so mask[p, f] = (f - 128d >= p)
    p = np.arange(128)[:, None]
    m = np.arange(896)[None, :]
    return ((m - 384) >= p).astype(np.float32)


def kernel(x, Wk, Wq, Wv, _trace=False, _trace_kwargs=None):
    global LAST_EXEC_TIME_NS, LAST_RESULTS
    x = np.ascontiguousarray(np.asarray(x, dtype=np.float32))
    Wk = np.asarray(Wk, dtype=np.float32)
    Wq = np.asarray(Wq, dtype=np.float32)
    Wv = np.asarray(Wv, dtype=np.float32)

    wqk = np.ascontiguousarray(
        np.concatenate([Wq.T, Wk.T], axis=1)).astype(ml_dtypes.bfloat16)  # [C, 128]
    wv = np.ascontiguousarray(Wv.T).astype(ml_dtypes.bfloat16)            # [C, H]
    mask = _make_mask().astype(ml_dtypes.bfloat16)
    ident = np.eye(128, dtype=np.float32)
    ident_bf = np.eye(64, dtype=ml_dtypes.bfloat16)
    ones_arr = np.zeros((128, NKC * 32), dtype=ml_dtypes.bfloat16)
    ones_arr[:, 0::32] = 1.0

    in_maps = []
    for core in range(NCORES):
        xb = x[BPC * core:BPC * (core + 1)]                 # [2, T, C]
        xtb = np.ascontiguousarray(xb.transpose(0, 2, 1)).astype(ml_dtypes.bfloat16)
        in_maps.append({"xt": xtb, "wqk": wqk, "wv": wv, "mask": mask,
                        "ident": ident, "ident_bf": ident_bf, "ones": ones_arr})

    nc = _get_nc()
    kwargs = {}
    if _trace:
        kwargs["trace"] = True
        if _trace_kwargs:
            kwargs.update(_trace_kwargs)
    res = bass_utils.run_bass_kernel_spmd(nc, in_maps, core_ids=list(range(NCORES)),
                                          **kwargs)
    LAST_EXEC_TIME_NS = res.exec_time_ns
    LAST_RESULTS = res

    out = np.empty((B, T, H), dtype=np.float32)
    for core in range(NCORES):
        out[BPC * core:BPC * (core + 1)] = res.results[core]["y"]
    return out



# revision 15
# speedup vs baseline: 1.0009x; 1.0009x over previous
"""Causal single-head attention (B=16, T=2048, C=1024, H=64) on 8 TRN2 NeuronCores.

v2 strategy (per core: 2 batches, weights replicated, batch-interleaved):
- Projections bf16, W-stationary, x^T moving -> QKT [128,T] (rows 0:64 Q^T,
  64:128 K^T) and VT [64,512] per slice; W prescaled by 32 on host.
- S^T = K_chunk^T-stationary @ Q^T bf16, pairs of k-chunks into one
  2-bank PSUM tile [128,1024].
- exp on Scalar over chunk pairs, scale=0.125/1024, bias=-4 (keeps P<240
  for fp8), output fp8 e4m3 (bf16 for j=0: accuracy guard for early rows).
- Causal mask: multiply only the diagonal 128/256-col blocks on DVE.
- V natural layout via XBAR transpose-DMA (bf16) -> vbig_bf; DVE cast to
  fp8 vbig8; col 64 of each 66-wide chunk = 32.0 (denominator ones row,
  cancels the W prescale).
- PV: fp8 DoubleRow over chunk pairs (contraction 256) -> O^T[66,512] PSUM;
  j=0 in bf16 singles. Row 64 = softmax denominator.
- Finalize: copy O^T to SBUF f32r, PE-transpose 128-col blocks, DVE
  reciprocal of denom col, DVE scale -> ybuf, one y DMA per batch.
"""
import os
import sys

for _p in ("/opt/trn_rl_repo", "/root/.axon_site/_ro/trn_rl_repo"):
    if os.path.isdir(_p) and _p not in sys.path:
        sys.path.insert(0, _p)

import numpy as np
import ml_dtypes
import concourse.bacc as bacc
import concourse.mybir as mybir
from concourse.tile import TileContext
from concourse import bass_utils

F32 = mybir.dt.float32
F32R = mybir.dt.float32r
BF16 = mybir.dt.bfloat16
FP8 = mybir.dt.float8e4
EXP = mybir.ActivationFunctionType.Exp
DR = mybir.MatmulPerfMode.DoubleRow

B, T, C, H = 16, 2048, 1024, 64
NCORES = 8
BPC = B // NCORES          # batches per core
NTS = T // 512             # 4 t/q slices of 512
NCH = C // 128             # 8 contraction chunks
NKC = T // 128             # 16 k chunks
VS = 80                    # vbig chunk stride (64 V + denom + pad; 16B-aligned for DoubleRow)
SW = 32.0                  # host prescale on W (keeps fp8 out of denormals)
SCALE = 0.125 / (SW * SW)  # exp scale: H**-0.5 / SW^2
EXPBIAS = -4.0             # max scaled S is 8.85 -> max P = e^4.85 ~ 128 < fp8 max 240

LAST_EXEC_TIME_NS = None
LAST_RESULTS = None


def build(debug=False):
    nc = bacc.Bacc(trn_type="TRN2")
    xt = nc.dram_tensor("xt", [BPC, C, T], BF16, kind="ExternalInput")
    dbg = {}
    if debug:
        dbg["qkt"] = nc.dram_tensor("dqkt", [128, T], BF16, kind="ExternalOutput")
        dbg["kt"] = nc.dram_tensor("dkt", [64, T], BF16, kind="ExternalOutput")
        dbg["v8"] = nc.dram_tensor("dv8", [128, NKC * VS], FP8,
                                   kind="ExternalOutput")
        dbg["vbf"] = nc.dram_tensor("dvbf", [128, 8 * VS], BF16,
                                    kind="ExternalOutput")
        dbg["p8"] = nc.dram_tensor("dp8", [128, 1024], FP8,
                                   kind="ExternalOutput")
        dbg["pbf"] = nc.dram_tensor("dpbf", [128, 1024], BF16,
                                    kind="ExternalOutput")
        dbg["osb"] = nc.dram_tensor("dosb", [VS, 512], F32R,
                                    kind="ExternalOutput")
    # const blobs: bf16 [128, 1024 wqk | 512 wv | 896 mask], fp8 mask, f32r id
    cbf = nc.dram_tensor("cbf", [128, 1024 + 512 + 896], BF16,
                         kind="ExternalInput")
    c8 = nc.dram_tensor("c8", [128, 896], FP8, kind="ExternalInput")
    cid = nc.dram_tensor("cid", [128, VS], F32R, kind="ExternalInput")
    y = nc.dram_tensor("y", [BPC, T, H], F32, kind="ExternalOutput")

    with TileContext(nc) as tc:
        with tc.tile_pool(name="const", bufs=1) as const, \
             tc.tile_pool(name="xpool", bufs=2 * NTS) as xpool, \
             tc.tile_pool(name="qktp", bufs=2) as qktp, \
             tc.tile_pool(name="ktp", bufs=2) as ktp, \
             tc.tile_pool(name="vtp", bufs=3) as vtp, \
             tc.tile_pool(name="vnp", bufs=3) as vnp, \
             tc.tile_pool(name="vbfp", bufs=2) as vbfp, \
             tc.tile_pool(name="v8p", bufs=2) as v8p, \
             tc.tile_pool(name="p8p", bufs=4) as p8p, \
             tc.tile_pool(name="pbfp", bufs=2) as pbfp, \
             tc.tile_pool(name="osbp", bufs=2) as osbp, \
             tc.tile_pool(name="ybp", bufs=2) as ybp, \
             tc.tile_pool(name="recp", bufs=8) as recp, \
             tc.tile_pool(name="ps_proj", bufs=2, space="PSUM") as ps_proj, \
             tc.tile_pool(name="ps_s", bufs=2, space="PSUM") as ps_s, \
             tc.tile_pool(name="ps_o", bufs=2, space="PSUM") as ps_o:

            # ---- constants: 3 DMAs ----
            cbf_sb = const.tile([128, 1024 + 512 + 896], BF16, name="cbf_sb")
            nc.sync.dma_start(cbf_sb[:], cbf[:])
            wqk_sb = cbf_sb[:, 0:1024]            # [128, 8*128] c-chunked
            wv_sb = cbf_sb[:, 1024:1536]          # [128, 8*64]
            mask_sb = cbf_sb[:, 1536:2432]        # [128, 896]
            mask8_sb = const.tile([128, 896], FP8, name="mask8_sb")
            nc.sync.dma_start(mask8_sb[:], c8[:])
            id_sb = const.tile([128, VS], F32R, name="id_sb")
            nc.sync.dma_start(id_sb[:], cid[:])
            bias_sb = const.tile([128, 1], F32, name="bias_sb")
            nc.vector.memset(bias_sb[:], EXPBIAS)

            # ---- prefetch all x slices ----
            xgs = {}
            for b in range(BPC):
                for ts in range(NTS):
                    xg = xpool.tile([128, NCH * 512], BF16,
                                    name=f"xg{b}_{ts}", tag="xg")
                    src = xt[b, :, 512 * ts:512 * (ts + 1)].rearrange(
                        "(a p) t -> p a t", p=128)
                    dst = xg[:].rearrange("p (a t) -> p a t", t=512)
                    eng = nc.sync if (b * NTS + ts) % 2 == 0 else nc.gpsimd
                    eng.dma_start(dst, src)
                    xgs[(b, ts)] = xg

            # ---- per-batch persistent tiles ----
            qkt = [qktp.tile([128, T], BF16, name=f"qkt{b}", tag="qkt")
                   for b in range(BPC)]
            kt = [ktp.tile([64, T], BF16, name=f"kt{b}", tag="kt")
                  for b in range(BPC)]
            vbf = [vbfp.tile([128, 8 * VS], BF16, name=f"vbf{b}", tag="vbf")
                   for b in range(BPC)]
            v8 = [v8p.tile([128, NKC * VS], FP8, name=f"v8{b}", tag="v8")
                  for b in range(BPC)]
            ybuf = [ybp.tile([128, NKC * H], F32, name=f"yb{b}", tag="yb")
                    for b in range(BPC)]
            for b in range(BPC):
                # zero V tiles, then set denominator column (=SW) per chunk
                nc.gpsimd.memset(v8[b][:], 0.0)
                nc.gpsimd.memset(
                    v8[b][:].rearrange("p (i c) -> p i c", c=VS)[:, :, H:H + 1],
                    SW)
                nc.gpsimd.memset(vbf[b][:], 0.0)
                nc.gpsimd.memset(
                    vbf[b][:].rearrange("p (i c) -> p i c", c=VS)[:, :, H:H + 1],
                    SW)

            # ---- main: batches interleaved at slice granularity ----
            for step in range(BPC * NTS):
                b, ts = step % BPC, step // BPC
                xg = xgs[(b, ts)]
                xts = [xg[:, 512 * c:512 * (c + 1)] for c in range(NCH)]

                # projections
                qk_ps = ps_proj.tile([128, 512], F32, name="qk_ps", tag="psp")
                for c in range(NCH):
                    nc.tensor.matmul(qk_ps[:], wqk_sb[:, 128 * c:128 * (c + 1)],
                                     xts[c], start=(c == 0), stop=(c == NCH - 1))
                nc.vector.tensor_copy(qkt[b][:, 512 * ts:512 * (ts + 1)], qk_ps[:])
                nc.gpsimd.dma_start(kt[b][:, 512 * ts:512 * (ts + 1)],
                                    qkt[b][64:128, 512 * ts:512 * (ts + 1)])
                v_ps = ps_proj.tile([64, 512], F32, name="v_ps", tag="psp")
                for c in range(NCH):
                    nc.tensor.matmul(v_ps[:], wv_sb[:, 64 * c:64 * (c + 1)],
                                     xts[c], start=(c == 0), stop=(c == NCH - 1))
                vt = vtp.tile([64, 512], BF16, name="vt", tag="vt")
                nc.scalar.copy(vt[:], v_ps[:])
                # V natural via XBAR transpose-DMA into contiguous staging
                vnat = vnp.tile([128, 256], BF16, name="vnat", tag="vnat")
                vnv = vnat[:].rearrange("p (i c) -> p i c", c=H)
                nc.sync.dma_start_transpose(vnv, vt[:])
                # fp8 cast into strided chunk layout (+ bf16 copy for j=0)
                v8dst = v8[b][:].rearrange("p (i c) -> p i c", c=VS)[
                    :, 4 * ts:4 * ts + 4, 0:H]
                nc.vector.tensor_copy(v8dst, vnv)
                if ts <= 1:
                    vbfdst = vbf[b][:].rearrange("p (i c) -> p i c", c=VS)[
                        :, 4 * ts:4 * ts + 4, 0:H]
                    nc.vector.tensor_copy(vbfdst, vnv)

                # ---- attention j == ts for batch b ----
                j = ts
                npair = 2 * j + 2
                o_ps = ps_o.tile([VS, 512], F32, name="o_ps", tag="pso")
                pend = None  # deferred PV (pipeline 1 pair behind)
                for i2 in range(npair):
                    i0 = 2 * i2
                    d0 = i0 - 4 * j
                    o = 128 * d0 if d0 > 0 else 0
                    w = 512 - o
                    s_ps = ps_s.tile([128, 1024], F32, name="s_ps", tag="pss")
                    mv = qkt[b][0:64, 512 * j + o:512 * (j + 1)]
                    nc.tensor.matmul(s_ps[:, o:512],
                                     kt[b][:, 128 * i0:128 * (i0 + 1)], mv,
                                     start=True, stop=True)
                    nc.tensor.matmul(s_ps[:, 512 + o:1024],
                                     kt[b][:, 128 * (i0 + 1):128 * (i0 + 2)], mv,
                                     start=True, stop=True)
                    if j <= 1:
                        ptile = pbfp.tile([128, 1024], BF16, name="pbf", tag="pbf")
                        msk = mask_sb
                    else:
                        ptile = p8p.tile([128, 1024], FP8, name="p8", tag="p8")
                        msk = mask8_sb
                    sv = s_ps[:].rearrange("p (c n) -> p c n", n=512)[:, :, o:512]
                    pv = ptile[:].rearrange("p (c n) -> p c n", n=512)[:, :, o:512]
                    nc.scalar.activation(pv, sv, EXP, bias=bias_sb[:], scale=SCALE)
                    p2 = ptile[:].rearrange("p (c n) -> p c n", n=512)
                    if d0 >= 0:   # diagonal pair: mask the partial blocks
                        nc.vector.tensor_mul(p2[:, 0, o:o + 128],
                                             p2[:, 0, o:o + 128],
                                             msk[:, 384:512])
                        nc.vector.tensor_mul(p2[:, 1, o:o + 256],
                                             p2[:, 1, o:o + 256],
                                             msk[:, 256:512])
                    if debug and b == 0 and j == 1 and i2 == 0:
                        nc.gpsimd.dma_start(dbg["p8"][:], ptile[:])
                    if debug and b == 0 and j == 0 and i2 == 0:
                        nc.gpsimd.dma_start(dbg["pbf"][:], ptile[:])
                    if pend is not None:
                        pend()
                    def mk_pv(i2=i2, i0=i0, o=o, ptile=ptile, j=j, b=b,
                              o_ps=o_ps, npair=npair):
                        def run():
                            first = (i2 == 0)
                            last = (i2 == npair - 1)
                            pm = ptile[:].rearrange("p (c n) -> p c n", n=512)
                            if j <= 1:
                                vb2 = vbf[b][:].rearrange(
                                    "p (i c) -> p i c", c=VS)
                                nc.tensor.matmul(
                                    o_ps[:, o:512], vb2[:, i0, :],
                                    pm[:, 0, o:512],
                                    start=first, stop=False)
                                nc.tensor.matmul(
                                    o_ps[:, o:512], vb2[:, i0 + 1, :],
                                    pm[:, 1, o:512],
                                    start=False, stop=last)
                            else:
                                v82 = v8[b][:].rearrange(
                                    "p (i c) -> p i c", c=VS)
                                nc.tensor.matmul(
                                    o_ps[:, o:512], v82[:, i0:i0 + 2, :],
                                    pm[:, :, o:512],
                                    start=first, stop=last, perf_mode=DR)
                        return run
                    pend = mk_pv()
                pend()

                # ---- finalize q-slice j: normalize + output ----
                o_sb = osbp.tile([VS, 512], F32R, name="o_sb", tag="osb")
                nc.vector.tensor_copy(o_sb[:], o_ps[:])
                f_all = ps_o.tile([128, 4 * VS], F32R, name="f_all", tag="pso")
                for s in range(4):
                    nc.tensor.transpose(f_all[:, VS * s:VS * (s + 1)],
                                        o_sb[:, 128 * s:128 * (s + 1)],
                                        id_sb[0:VS, 0:VS])
                for s in range(4):
                    rec = recp.tile([128, 1], F32, name="rec", tag="rec")
                    nc.vector.reciprocal(rec[:], f_all[:, VS * s + H:VS * s + H + 1])
                    nc.vector.tensor_scalar_mul(
                        ybuf[b][:, H * (4 * j + s):H * (4 * j + s + 1)],
                        f_all[:, VS * s:VS * s + H], rec[:])
                if debug and b == 0 and j == 3:
                    nc.gpsimd.dma_start(dbg["osb"][:], o_sb[:])
                if ts == NTS - 1:
                    if debug and b == 0:
                        nc.gpsimd.dma_start(dbg["qkt"][:], qkt[0][:])
                        nc.gpsimd.dma_start(dbg["kt"][:], kt[0][:])
                        nc.gpsimd.dma_start(dbg["v8"][:], v8[0][:])
                        nc.gpsimd.dma_start(dbg["vbf"][:], vbf[0][:])
                    ydst = y[b].rearrange("(i p) h -> p i h", p=128)
                    ysrc = ybuf[b][:].rearrange("p (i h) -> p i h", h=H)
                    nc.gpsimd.dma_start(ydst, ysrc)

    nc.finalize()
    return nc


_NC_CACHE = None


def _get_nc():
    global _NC_CACHE
    if _NC_CACHE is None:
        _NC_CACHE = build()
    return _NC_CACHE


def _make_mask():
    # mask[p, m] = 1.0 iff (m - 384) >= p
    p = np.arange(128)[:, None]
    m = np.arange(896)[None, :]
    return ((m - 384) >= p).astype(np.float32)


def kernel(x, Wk, Wq, Wv, _trace=False, _trace_kwargs=None):
    global LAST_EXEC_TIME_NS, LAST_RESULTS
    x = np.ascontiguousarray(np.asarray(x, dtype=np.float32))
    Wk = np.asarray(Wk, dtype=np.float32)
    Wq = np.asarray(Wq, dtype=np.float32)
    Wv = np.asarray(Wv, dtype=np.float32)

    # W prescaled by SW; packed per 128-row c-chunk: [128, chunk, m]
    wqk = np.concatenate([Wq.T, Wk.T], axis=1) * SW          # [C, 128]
    wqk_p = wqk.reshape(NCH, 128, 128).transpose(1, 0, 2).reshape(128, 1024)
    wv = Wv.T * SW                                           # [C, 64]
    wv_p = wv.reshape(NCH, 128, 64).transpose(1, 0, 2).reshape(128, 512)
    mask = _make_mask()
    cbf = np.concatenate([wqk_p, wv_p, mask], axis=1).astype(ml_dtypes.bfloat16)
    c8 = mask.astype(ml_dtypes.float8_e4m3fn)
    cid = np.zeros((128, VS), dtype=np.float32)
    cid[:VS, :VS] = np.eye(VS, dtype=np.float32)

    in_maps = []
    for core in range(NCORES):
        xb = x[BPC * core:BPC * (core + 1)]                  # [2, T, C]
        xtb = np.ascontiguousarray(xb.transpose(0, 2, 1)).astype(
            ml_dtypes.bfloat16)
        in_maps.append({"xt": xtb, "cbf": cbf, "c8": c8, "cid": cid})

    nc = _get_nc()
    kwargs = {}
    if _trace:
        kwargs["trace"] = True
        if _trace_kwargs:
            kwargs.update(_trace_kwargs)
    res = bass_utils.run_bass_kernel_spmd(nc, in_maps, core_ids=list(range(NCORES)),
                                          **kwargs)
    LAST_EXEC_TIME_NS = res.exec_time_ns
    LAST_RESULTS = res

    out = np.empty((B, T, H), dtype=np.float32)
    for core in range(NCORES):
        out[BPC * core:BPC * (core + 1)] = res.results[core]["y"]
    return out
